# revision 6
# baseline (speedup 1.0000x reference)
"""Trainium2 Bass kernel for the sparse-attention scores module.

Computes, for each batch b:
    scores[b, :] = softmax_s( v . tanh(W1 @ static[b] + W2 @ dynamic[b] + W3 @ hidden[b]) )
with W = [W1 | W2 | W3] of shape [H, 3H], static/dynamic [B, H, S], hidden [B, H].

Sharding: data-parallel over B across 8 NeuronCores (8 batches per core).
Per core the kernel streams 64 MiB of encoder data from HBM (memory-bound),
runs the two [256,256]@[256,4096] matmuls per batch in fp32r on the PE,
adds the per-batch bias and applies tanh on the ACT engine, and reduces with
v via a masked-lhsT PE matmul whose [8,512] PSUM output lands each batch's
scores directly on partition b.  A DVE add merges chunks into a [8,4096]
scores tile, so no SBUF->SBUF scatter DMAs are needed and the softmax
epilogue is a per-partition exp/normalize.
"""

import sys

sys.path.insert(0, "/opt/trn_rl_repo")

import numpy as np

B, H, S = 64, 256, 4096
N_CORES = 8
BPC = B // N_CORES          # batches per core
KK = H // 128               # 2 contraction chunks
MM = H // 128               # 2 output-row chunks
NCH = S // 512              # 8 psum column chunks
NQ = 4                      # input DMA quarters along s
SQ = S // NQ                # 1024 columns per quarter


def build_bass(reps: int = 1, loop_iters: int = 0):
    """Build the per-core Bass program. reps>1 unrolls the whole computation
    multiple times; loop_iters>0 additionally wraps the unrolled body in a
    hardware loop. Both are used only for timing by differencing."""
    import contextlib

    import concourse.bacc as bacc
    import concourse.tile as tile
    from concourse import mybir

    f32 = mybir.dt.float32
    f32r = mybir.dt.float32r

    nc = bacc.Bacc(None)

    # Quarter-major host-packed layout: each (b, q) slice is one fully
    # contiguous 2 MiB region read sequentially from HBM.
    xc = nc.dram_tensor("xc", [BPC, NQ, 128, 2, KK, SQ], f32r, kind="ExternalInput")
    wt = nc.dram_tensor("wt", [128, 12, 128], f32r, kind="ExternalInput")
    ht = nc.dram_tensor("ht", [128, KK, BPC], f32r, kind="ExternalInput")
    vm = nc.dram_tensor("vm", [128, MM, BPC, BPC], f32r, kind="ExternalInput")
    out = nc.dram_tensor("out", [BPC, S], f32, kind="ExternalOutput")

    with tile.TileContext(nc) as tc:
        with (
            tc.tile_pool(name="consts", bufs=1) as consts,
            tc.tile_pool(name="xpool", bufs=6) as xpool,
            tc.tile_pool(name="tpool", bufs=6) as tpool,
            tc.tile_pool(name="spool", bufs=2) as spool,
            tc.tile_pool(name="mpsum", bufs=4, space="PSUM") as mpsum,
            tc.tile_pool(name="vpsum", bufs=2, space="PSUM") as vpsum,
            tc.tile_pool(name="spsum", bufs=2, space="PSUM") as spsum,
        ):
            wt_sb = consts.tile([128, 12, 128], f32r)
            nc.sync.dma_start(out=wt_sb, in_=wt[:, :, :])
            ht_sb = consts.tile([128, KK, BPC], f32r)
            nc.sync.dma_start(out=ht_sb, in_=ht[:, :, :])
            vm_sb = consts.tile([128, MM, BPC, BPC], f32r)
            nc.sync.dma_start(out=vm_sb, in_=vm[:, :, :, :])

            # Per-batch bias: bias[m*128+h', b] = (W3 @ hidden[b])[m*128+h']
            bias_sb = consts.tile([128, MM, BPC], f32)
            for m in range(MM):
                bias_ps = spsum.tile([128, BPC], f32, tag="small")
                for kk in range(KK):
                    nc.tensor.matmul(
                        bias_ps,
                        lhsT=wt_sb[:, 8 + kk * 2 + m, :],
                        rhs=ht_sb[:, kk, :],
                        start=(kk == 0),
                        stop=(kk == KK - 1),
                    )
                nc.vector.tensor_copy(out=bias_sb[:, m, :], in_=bias_ps)

            loop_cm = (
                tc.For_i(0, loop_iters, 1) if loop_iters else contextlib.nullcontext()
            )
            with loop_cm:
              for _ in range(reps):
                # Scores live as [8, 4096] with partition p = batch b, so the
                # epilogue is a plain per-partition exp/normalize and the
                # output DMA is contiguous.
                scores8 = spool.tile([BPC, S], f32, tag="scores")
                esums = spool.tile([BPC, NCH], f32, tag="esums")
                pending = None

                def emit_vdot(pend):
                    # v-dot runs one chunk late so the tanh results are
                    # ready and the PE never waits on the ACT engine.  The
                    # lhsT is v masked into column b, so the [8,512] PSUM
                    # result lands on partition b directly (rows != b are
                    # exact zeros) and a DVE add/copy merges it into the
                    # scores tile -- no scatter DMA.
                    b, n, vp, tts = pend
                    for m in range(MM):
                        nc.tensor.matmul(
                            vp,
                            lhsT=vm_sb[:, m, b, :],
                            rhs=tts[m],
                            start=(m == 0),
                            stop=(m == MM - 1),
                        )
                    sl = scores8[:, n * 512 : (n + 1) * 512]
                    if b == 0:
                        nc.vector.tensor_copy(out=sl, in_=vp)
                    else:
                        nc.vector.tensor_add(out=sl, in0=vp, in1=sl)
                    if b == BPC - 1:
                        # Last batch for this column chunk: exponentiate in
                        # place while later chunks still stream.  Scores are
                        # small (|s| < ~6), so skip the max subtraction.
                        nc.scalar.activation(
                            out=sl,
                            in_=sl,
                            func=mybir.ActivationFunctionType.Exp,
                            accum_out=esums[:, n : n + 1],
                        )

                for b in range(BPC):
                    # Stream both encoder tensors in 2 MiB quarters so the
                    # PE can start each pair of column chunks as soon as its
                    # slice of data lands.
                    xq = []
                    for q in range(NQ):
                        xt = xpool.tile([128, 2, KK, SQ], f32r, tag="xq")
                        nc.sync.dma_start(out=xt, in_=xc[b, q])
                        xq.append(xt)

                    for n in range(NCH):
                        q, r = divmod(n, NCH // NQ)
                        tts = []
                        for m in range(MM):
                            ps = mpsum.tile([128, 512], f32, tag="ps")
                            i = 0
                            for t in range(2):
                                for kk in range(KK):
                                    nc.tensor.matmul(
                                        ps,
                                        lhsT=wt_sb[:, t * 4 + kk * 2 + m, :],
                                        rhs=xq[q][:, t, kk, r * 512 : (r + 1) * 512],
                                        start=(i == 0),
                                        stop=(i == 3),
                                    )
                                    i += 1
                            tt = tpool.tile([128, 512], f32r, tag="tt")
                            nc.scalar.activation(
                                out=tt,
                                in_=ps,
                                func=mybir.ActivationFunctionType.Tanh,
                                bias=bias_sb[:, m, b : b + 1],
                                scale=1.0,
                            )
                            tts.append(tt)
                        if pending is not None:
                            emit_vdot(pending)
                        vp = vpsum.tile([BPC, 512], f32, tag="vp")
                        pending = (b, n, vp, tts)
                # flush the last batch's final v-dot (and its exp) after the loop
                emit_vdot(pending)
                pending = None

                # Softmax normalization: per-batch sum of the chunk exp-sums,
                # reciprocal, then scale.  The multiplies are split across
                # DVE / ACT / GPSIMD so the tail is short.
                bsum = spool.tile([BPC, 1], f32, tag="bsum")
                nc.vector.reduce_sum(out=bsum, in_=esums, axis=mybir.AxisListType.X)
                recip = spool.tile([BPC, 1], f32, tag="recip")
                nc.vector.reciprocal(out=recip, in_=bsum)
                nc.vector.tensor_scalar_mul(
                    out=scores8[:, 0:2048], in0=scores8[:, 0:2048], scalar1=recip
                )
                nc.scalar.activation(
                    out=scores8[:, 2048:3328],
                    in_=scores8[:, 2048:3328],
                    func=mybir.ActivationFunctionType.Copy,
                    scale=recip,
                )
                nc.gpsimd.tensor_scalar_mul(
                    out=scores8[:, 3328:4096], in0=scores8[:, 3328:4096], scalar1=recip
                )
                # Output DMA rides the (idle) gpsimd queue so its wait on the
                # epilogue never blocks the next rep's input stream on the
                # sync queue.
                nc.gpsimd.dma_start(out=out[:, :], in_=scores8)

    nc.finalize()
    return nc


def prep_shared_inputs(W: np.ndarray, v: np.ndarray, decoder_hidden: np.ndarray):
    """Host-side layout marshaling of the small replicated parameters."""
    W = np.ascontiguousarray(W, dtype=np.float32)
    wt_tiles = np.empty((128, 12, 128), np.float32)
    for t in range(3):
        Wt = W[:, t * H : (t + 1) * H].T  # [k, h]
        for kk in range(KK):
            for m in range(MM):
                j = t * 4 + kk * 2 + m
                wt_tiles[:, j, :] = Wt[kk * 128 : (kk + 1) * 128, m * 128 : (m + 1) * 128]
    vt = np.ascontiguousarray(v[0].reshape(KK, 128).T, dtype=np.float32)  # [p, m]
    vm_tiles = np.zeros((128, MM, BPC, BPC), np.float32)
    for b in range(BPC):
        vm_tiles[:, :, b, b] = vt
    hT = decoder_hidden[0].T.astype(np.float32)  # [H, B]
    return wt_tiles, vm_tiles, hT


_CACHED = {}


def _get_nc(reps: int = 1, loop_iters: int = 0):
    key = (reps, loop_iters)
    if key not in _CACHED:
        _CACHED[key] = build_bass(reps, loop_iters)
    return _CACHED[key]


def make_in_maps(static_enc, dynamic_enc, decoder_hidden, W, v):
    wt_tiles, vm_tiles, hT = prep_shared_inputs(W, v, decoder_hidden)
    static_enc = np.ascontiguousarray(static_enc, dtype=np.float32)
    dynamic_enc = np.ascontiguousarray(dynamic_enc, dtype=np.float32)
    in_maps = []
    for c in range(N_CORES):
        b0 = c * BPC
        ht_c = np.ascontiguousarray(
            hT[:, b0 : b0 + BPC].reshape(KK, 128, BPC).transpose(1, 0, 2)
        )  # [p, kk, b]
        # [b, t, kk, p, q, s'] -> quarter-major [b, q, p, t, kk, s'] so each
        # (b, q) slab is read as one sequential 2 MiB HBM stream.
        xc_c = np.ascontiguousarray(
            np.stack(
                [static_enc[b0 : b0 + BPC], dynamic_enc[b0 : b0 + BPC]], axis=1
            )
            .reshape(BPC, 2, KK, 128, NQ, SQ)
            .transpose(0, 4, 3, 1, 2, 5)
        )  # [b, q, p, t, kk, s']
        in_maps.append(
            {
                "xc": xc_c,
                "wt": wt_tiles,
                "ht": ht_c,
                "vm": vm_tiles,
            }
        )
    return in_maps


def kernel(static_enc, dynamic_enc, decoder_hidden, W, v):
    from concourse.bass_utils import run_bass_kernel_spmd

    nc = _get_nc(reps=1)
    in_maps = make_in_maps(static_enc, dynamic_enc, decoder_hidden, W, v)
    res = run_bass_kernel_spmd(nc, in_maps, core_ids=list(range(N_CORES)))
    return np.concatenate([r["out"] for r in res.results], axis=0)


# revision 18
# speedup vs baseline: 1.3458x; 1.3458x over previous
"""Trainium2 Bass kernel for the sparse-attention scores module.

Computes, for each batch b:
    scores[b, :] = softmax_s( v . tanh(W1 @ static[b] + W2 @ dynamic[b] + W3 @ hidden[b]) )
with W = [W1 | W2 | W3] of shape [H, 3H], static/dynamic [B, H, S], hidden [B, H].

Sharding: data-parallel over B across 8 NeuronCores (8 batches per core).
Per core the kernel streams 64 MiB of encoder data from HBM (memory-bound),
runs the two [256,256]@[256,4096] matmuls per batch in fp32r on the PE,
adds the per-batch bias and applies tanh on the ACT engine, and reduces with
v via a masked-lhsT PE matmul whose [8,512] PSUM output lands each batch's
scores directly on partition b.  A DVE add merges chunks into a [8,4096]
scores tile, so no SBUF->SBUF scatter DMAs are needed and the softmax
epilogue is a per-partition exp/normalize.
"""

import sys

sys.path.insert(0, "/opt/trn_rl_repo")

import ml_dtypes
import numpy as np

BF16 = ml_dtypes.bfloat16

B, H, S = 64, 256, 4096
N_CORES = 8
BPC = B // N_CORES          # batches per core
KK = H // 128               # 2 contraction chunks
MM = H // 128               # 2 output-row chunks
NCH = S // 512              # 8 psum column chunks
NQ = 4                      # input DMA quarters along s
SQ = S // NQ                # 1024 columns per quarter


def build_bass(reps: int = 1, loop_iters: int = 0, nq: int = NQ):
    """Build the per-core Bass program. reps>1 unrolls the whole computation
    multiple times; loop_iters>0 additionally wraps the unrolled body in a
    hardware loop. Both are used only for timing by differencing."""
    import contextlib

    import concourse.bacc as bacc
    import concourse.tile as tile
    from concourse import mybir

    f32 = mybir.dt.float32
    f32r = mybir.dt.float32r
    bf16 = mybir.dt.bfloat16

    nc = bacc.Bacc(None)

    # Encoder streams (the HBM bottleneck) and the tiny W/hidden params are
    # staged as bf16 (host-cast): halves the 64 MiB/core stream at a ~1.4e-3
    # rel-l2 cost, far under the 2e-2 gate.  tanh outputs and the v-dot stay
    # fp32.
    xc = nc.dram_tensor("xc", [BPC, 2, H, S], bf16, kind="ExternalInput")
    wt = nc.dram_tensor("wt", [128, 12, 128], bf16, kind="ExternalInput")
    ht = nc.dram_tensor("ht", [128, KK, BPC], bf16, kind="ExternalInput")
    vm = nc.dram_tensor("vm", [128, MM, BPC, BPC], f32r, kind="ExternalInput")
    out = nc.dram_tensor("out", [BPC, S], f32, kind="ExternalOutput")

    sq = S // nq
    with tile.TileContext(nc) as tc:
        with (
            tc.tile_pool(name="consts", bufs=1) as consts,
            tc.tile_pool(name="xpool", bufs=(6 if nq >= 4 else 3)) as xpool,
            tc.tile_pool(name="tpool", bufs=6) as tpool,
            tc.tile_pool(name="spool", bufs=2) as spool,
            tc.tile_pool(name="mpsum", bufs=4, space="PSUM") as mpsum,
            tc.tile_pool(name="vpsum", bufs=2, space="PSUM") as vpsum,
            tc.tile_pool(name="spsum", bufs=2, space="PSUM") as spsum,
        ):
            wt_sb = consts.tile([128, 12, 128], bf16)
            nc.sync.dma_start(out=wt_sb, in_=wt[:, :, :])
            ht_sb = consts.tile([128, KK, BPC], bf16)
            nc.sync.dma_start(out=ht_sb, in_=ht[:, :, :])
            vm_sb = consts.tile([128, MM, BPC, BPC], f32r)
            nc.sync.dma_start(out=vm_sb, in_=vm[:, :, :, :])

            # Per-batch bias: bias[m*128+h', b] = (W3 @ hidden[b])[m*128+h']
            bias_sb = consts.tile([128, MM, BPC], f32)
            for m in range(MM):
                bias_ps = spsum.tile([128, BPC], f32, tag="small")
                for kk in range(KK):
                    nc.tensor.matmul(
                        bias_ps,
                        lhsT=wt_sb[:, 8 + kk * 2 + m, :],
                        rhs=ht_sb[:, kk, :],
                        start=(kk == 0),
                        stop=(kk == KK - 1),
                    )
                nc.vector.tensor_copy(out=bias_sb[:, m, :], in_=bias_ps)

            loop_cm = (
                tc.For_i(0, loop_iters, 1) if loop_iters else contextlib.nullcontext()
            )
            with loop_cm:
              for _ in range(reps):
                # Scores live as [8, 4096] with partition p = batch b, so the
                # epilogue is a plain per-partition exp/normalize and the
                # output DMA is contiguous.
                scores8 = spool.tile([BPC, S], f32, tag="scores")
                esums = spool.tile([BPC, NCH], f32, tag="esums")
                pending = None

                def emit_vdot(pend):
                    # v-dot runs one chunk late so the tanh results are
                    # ready and the PE never waits on the ACT engine.  The
                    # lhsT is v masked into column b, so the [8,512] PSUM
                    # result lands on partition b directly (rows != b are
                    # exact zeros) and a DVE add/copy merges it into the
                    # scores tile -- no scatter DMA.
                    b, n, vp, tts = pend
                    for m in range(MM):
                        nc.tensor.matmul(
                            vp,
                            lhsT=vm_sb[:, m, b, :],
                            rhs=tts[m],
                            start=(m == 0),
                            stop=(m == MM - 1),
                        )
                    sl = scores8[:, n * 512 : (n + 1) * 512]
                    if b == 0:
                        nc.vector.tensor_copy(out=sl, in_=vp)
                    else:
                        nc.vector.tensor_add(out=sl, in0=vp, in1=sl)
                    if b == BPC - 1:
                        # Last batch for this column chunk: exponentiate in
                        # place while later chunks still stream.  Scores are
                        # small (|s| < ~6), so skip the max subtraction.
                        nc.scalar.activation(
                            out=sl,
                            in_=sl,
                            func=mybir.ActivationFunctionType.Exp,
                            accum_out=esums[:, n : n + 1],
                        )

                for b in range(BPC):
                    # Stream both encoder tensors in 2 MiB quarters so the
                    # PE can start each pair of column chunks as soon as its
                    # slice of data lands.
                    xq = []
                    for q in range(nq):
                        xt = xpool.tile([128, 2, KK, sq], bf16, tag="xq")
                        src = xc[b, :, :, q * sq : (q + 1) * sq].rearrange(
                            "t (kk p) s -> p t kk s", p=128
                        )
                        nc.sync.dma_start(out=xt, in_=src)
                        xq.append(xt)

                    for n in range(NCH):
                        q, r = divmod(n, NCH // nq)
                        tts = []
                        for m in range(MM):
                            ps = mpsum.tile([128, 512], f32, tag="ps")
                            i = 0
                            for t in range(2):
                                for kk in range(KK):
                                    nc.tensor.matmul(
                                        ps,
                                        lhsT=wt_sb[:, t * 4 + kk * 2 + m, :],
                                        rhs=xq[q][:, t, kk, r * 512 : (r + 1) * 512],
                                        start=(i == 0),
                                        stop=(i == 3),
                                    )
                                    i += 1
                            tt = tpool.tile([128, 512], f32r, tag="tt")
                            nc.scalar.activation(
                                out=tt,
                                in_=ps,
                                func=mybir.ActivationFunctionType.Tanh,
                                bias=bias_sb[:, m, b : b + 1],
                                scale=1.0,
                            )
                            tts.append(tt)
                        if pending is not None:
                            emit_vdot(pending)
                        vp = vpsum.tile([BPC, 512], f32, tag="vp")
                        pending = (b, n, vp, tts)
                # flush the last batch's final v-dot (and its exp) after the loop
                emit_vdot(pending)
                pending = None

                # Softmax normalization: per-batch sum of the chunk exp-sums,
                # reciprocal, then scale.  The multiplies are split across
                # DVE / ACT / GPSIMD so the tail is short.
                bsum = spool.tile([BPC, 1], f32, tag="bsum")
                nc.vector.reduce_sum(out=bsum, in_=esums, axis=mybir.AxisListType.X)
                recip = spool.tile([BPC, 1], f32, tag="recip")
                nc.vector.reciprocal(out=recip, in_=bsum)
                nc.vector.tensor_scalar_mul(
                    out=scores8[:, 0:2048], in0=scores8[:, 0:2048], scalar1=recip
                )
                nc.scalar.activation(
                    out=scores8[:, 2048:3328],
                    in_=scores8[:, 2048:3328],
                    func=mybir.ActivationFunctionType.Copy,
                    scale=recip,
                )
                nc.gpsimd.tensor_scalar_mul(
                    out=scores8[:, 3328:4096], in0=scores8[:, 3328:4096], scalar1=recip
                )
                # Output DMA rides the (idle) gpsimd queue so its wait on the
                # epilogue never blocks the next rep's input stream on the
                # sync queue.
                nc.gpsimd.dma_start(out=out[:, :], in_=scores8)

    nc.finalize()
    return nc


def prep_shared_inputs(W: np.ndarray, v: np.ndarray, decoder_hidden: np.ndarray):
    """Host-side layout marshaling of the small replicated parameters."""
    W = np.ascontiguousarray(W, dtype=np.float32)
    wt_tiles = np.empty((128, 12, 128), np.float32)
    for t in range(3):
        Wt = W[:, t * H : (t + 1) * H].T  # [k, h]
        for kk in range(KK):
            for m in range(MM):
                j = t * 4 + kk * 2 + m
                wt_tiles[:, j, :] = Wt[kk * 128 : (kk + 1) * 128, m * 128 : (m + 1) * 128]
    vt = np.ascontiguousarray(v[0].reshape(KK, 128).T, dtype=np.float32)  # [p, m]
    vm_tiles = np.zeros((128, MM, BPC, BPC), np.float32)
    for b in range(BPC):
        vm_tiles[:, :, b, b] = vt
    hT = decoder_hidden[0].T.astype(np.float32)  # [H, B]
    return wt_tiles.astype(BF16), vm_tiles, hT


_CACHED = {}


def _get_nc(reps: int = 1, loop_iters: int = 0, nq: int = NQ):
    key = (reps, loop_iters, nq)
    if key not in _CACHED:
        _CACHED[key] = build_bass(reps, loop_iters, nq)
    return _CACHED[key]


def make_in_maps(static_enc, dynamic_enc, decoder_hidden, W, v):
    wt_tiles, vm_tiles, hT = prep_shared_inputs(W, v, decoder_hidden)
    static_enc = np.ascontiguousarray(static_enc, dtype=np.float32)
    dynamic_enc = np.ascontiguousarray(dynamic_enc, dtype=np.float32)
    in_maps = []
    for c in range(N_CORES):
        b0 = c * BPC
        ht_c = np.ascontiguousarray(
            hT[:, b0 : b0 + BPC].reshape(KK, 128, BPC).transpose(1, 0, 2)
        ).astype(BF16)  # [p, kk, b]
        xc_c = np.ascontiguousarray(
            np.stack(
                [static_enc[b0 : b0 + BPC], dynamic_enc[b0 : b0 + BPC]], axis=1
            ).astype(BF16)
        )  # [b, t, h, s]
        in_maps.append(
            {
                "xc": xc_c,
                "wt": wt_tiles,
                "ht": ht_c,
                "vm": vm_tiles,
            }
        )
    return in_maps


def kernel(static_enc, dynamic_enc, decoder_hidden, W, v):
    from concourse.bass_utils import run_bass_kernel_spmd

    nc = _get_nc(reps=1)
    in_maps = make_in_maps(static_enc, dynamic_enc, decoder_hidden, W, v)
    res = run_bass_kernel_spmd(nc, in_maps, core_ids=list(range(N_CORES)))
    return np.concatenate([r["out"] for r in res.results], axis=0)


# revision 27
# speedup vs baseline: 1.3579x; 1.0090x over previous
"""Trainium2 Bass kernel for the sparse-attention scores module.

Computes, for each batch b:
    scores[b, :] = softmax_s( v . tanh(W1 @ static[b] + W2 @ dynamic[b] + W3 @ hidden[b]) )
with W = [W1 | W2 | W3] of shape [H, 3H], static/dynamic [B, H, S], hidden [B, H].

Sharding: data-parallel over B across 8 NeuronCores (8 batches per core).
Per core the kernel streams 32 MiB of host-cast bf16 encoder data from HBM,
runs the two [256,256]@[256,4096] matmuls per batch in bf16 on the PE
(fp32 PSUM), adds the per-batch bias and applies tanh on the ACT engine,
and reduces with v via a masked-lhsT PE matmul whose [8,512] PSUM output
lands each batch's scores directly on partition b.  A DVE add merges chunks
into a [8,4096] scores tile, so no SBUF->SBUF scatter DMAs are needed and
the softmax epilogue is a per-partition exp/normalize with the multiplies
split across DVE/ACT/GPSIMD.  The bf16 staging costs ~1.4e-3 rel-l2
(gate: 2e-2); with it the kernel is PE-bound (~137 us of matmul per pass)
rather than HBM-bound.
"""

import sys

sys.path.insert(0, "/opt/trn_rl_repo")

import ml_dtypes
import numpy as np

BF16 = ml_dtypes.bfloat16

B, H, S = 64, 256, 4096
N_CORES = 8
BPC = B // N_CORES          # batches per core
KK = H // 128               # 2 contraction chunks
MM = H // 128               # 2 output-row chunks
NCH = S // 512              # 8 psum column chunks
NQ = 4                      # input DMA quarters along s
SQ = S // NQ                # 1024 columns per quarter


def build_bass(reps: int = 1, loop_iters: int = 0, nq: int = NQ, variant: str = "base"):
    """Build the per-core Bass program. reps>1 unrolls the whole computation
    multiple times; loop_iters>0 additionally wraps the unrolled body in a
    hardware loop. Both are used only for timing by differencing."""
    import contextlib

    import concourse.bacc as bacc
    import concourse.tile as tile
    from concourse import mybir

    f32 = mybir.dt.float32
    f32r = mybir.dt.float32r
    bf16 = mybir.dt.bfloat16

    nc = bacc.Bacc(None)

    # Encoder streams (the HBM bottleneck) and the tiny W/hidden params are
    # staged as bf16 (host-cast): halves the 64 MiB/core stream at a ~1.4e-3
    # rel-l2 cost, far under the 2e-2 gate.  tanh outputs and the v-dot stay
    # fp32.
    xc = nc.dram_tensor("xc", [BPC, 2, H, S], bf16, kind="ExternalInput")
    wt = nc.dram_tensor("wt", [128, 12, 128], bf16, kind="ExternalInput")
    ht = nc.dram_tensor("ht", [128, KK, BPC], bf16, kind="ExternalInput")
    vm = nc.dram_tensor("vm", [128, MM, BPC, BPC], f32r, kind="ExternalInput")
    out = nc.dram_tensor("out", [BPC, S], f32, kind="ExternalOutput")

    sq = S // nq
    with tile.TileContext(nc) as tc:
        with (
            tc.tile_pool(name="consts", bufs=1) as consts,
            tc.tile_pool(name="xpool", bufs=(6 if nq >= 4 else 3)) as xpool,
            tc.tile_pool(name="tpool", bufs=(8 if variant == "lag2" else 6)) as tpool,
            tc.tile_pool(name="spool", bufs=2) as spool,
            tc.tile_pool(
                name="mpsum", bufs=(5 if variant == "reorder" else 4), space="PSUM"
            ) as mpsum,
            tc.tile_pool(
                name="vpsum", bufs=(3 if variant == "lag2" else 2), space="PSUM"
            ) as vpsum,
            tc.tile_pool(
                name="spsum",
                bufs=(1 if variant in ("reorder", "lag2") else 2),
                space="PSUM",
            ) as spsum,
        ):
            wt_sb = consts.tile([128, 12, 128], bf16)
            nc.sync.dma_start(out=wt_sb, in_=wt[:, :, :])
            ht_sb = consts.tile([128, KK, BPC], bf16)
            nc.sync.dma_start(out=ht_sb, in_=ht[:, :, :])
            vm_sb = consts.tile([128, MM, BPC, BPC], f32r)
            nc.sync.dma_start(out=vm_sb, in_=vm[:, :, :, :])

            # Per-batch bias: bias[m*128+h', b] = (W3 @ hidden[b])[m*128+h']
            bias_sb = consts.tile([128, MM, BPC], f32)
            for m in range(MM):
                bias_ps = spsum.tile([128, BPC], f32, tag="small")
                for kk in range(KK):
                    nc.tensor.matmul(
                        bias_ps,
                        lhsT=wt_sb[:, 8 + kk * 2 + m, :],
                        rhs=ht_sb[:, kk, :],
                        start=(kk == 0),
                        stop=(kk == KK - 1),
                    )
                nc.vector.tensor_copy(out=bias_sb[:, m, :], in_=bias_ps)

            loop_cm = (
                tc.For_i(0, loop_iters, 1) if loop_iters else contextlib.nullcontext()
            )
            with loop_cm:
              for _ in range(reps):
                # Scores live as [8, 4096] with partition p = batch b, so the
                # epilogue is a plain per-partition exp/normalize and the
                # output DMA is contiguous.
                scores8 = spool.tile([BPC, S], f32, tag="scores")
                esums = spool.tile([BPC, NCH], f32, tag="esums")
                pending = []

                def emit_vdot(pend):
                    # v-dot runs one chunk late so the tanh results are
                    # ready and the PE never waits on the ACT engine.  The
                    # lhsT is v masked into column b, so the [8,512] PSUM
                    # result lands on partition b directly (rows != b are
                    # exact zeros) and a DVE add/copy merges it into the
                    # scores tile -- no scatter DMA.
                    b, n, vp, tts = pend
                    for m in range(MM):
                        nc.tensor.matmul(
                            vp,
                            lhsT=vm_sb[:, m, b, :],
                            rhs=tts[m],
                            start=(m == 0),
                            stop=(m == MM - 1),
                        )
                    sl = scores8[:, n * 512 : (n + 1) * 512]
                    if b == 0:
                        nc.vector.tensor_copy(out=sl, in_=vp)
                    else:
                        nc.vector.tensor_add(out=sl, in0=vp, in1=sl)
                    if b == BPC - 1:
                        # Last batch for this column chunk: exponentiate in
                        # place while later chunks still stream.  Scores are
                        # small (|s| < ~6), so skip the max subtraction.
                        nc.scalar.activation(
                            out=sl,
                            in_=sl,
                            func=mybir.ActivationFunctionType.Exp,
                            accum_out=esums[:, n : n + 1],
                        )

                for b in range(BPC):
                    # Stream both encoder tensors in 2 MiB quarters so the
                    # PE can start each pair of column chunks as soon as its
                    # slice of data lands.
                    xq = []
                    for q in range(nq):
                        xt = xpool.tile([128, 2, KK, sq], bf16, tag="xq")
                        src = xc[b, :, :, q * sq : (q + 1) * sq].rearrange(
                            "t (kk p) s -> p t kk s", p=128
                        )
                        nc.sync.dma_start(out=xt, in_=src)
                        xq.append(xt)

                    if variant == "reorder":
                        # Weight-stationary: each W tile multiplies all
                        # 512-col chunks of a quarter back-to-back, so the PE
                        # array reloads weights less often.
                        rq = NCH // nq
                        for q in range(nq):
                            tts_r = [[None] * MM for _ in range(rq)]
                            for m in range(MM):
                                pss = [
                                    mpsum.tile([128, 512], f32, tag="ps", name="ps")
                                    for _ in range(rq)
                                ]
                                tk = 0
                                for t in range(2):
                                    for kk in range(KK):
                                        for r in range(rq):
                                            nc.tensor.matmul(
                                                pss[r],
                                                lhsT=wt_sb[:, t * 4 + kk * 2 + m, :],
                                                rhs=xq[q][
                                                    :, t, kk, r * 512 : (r + 1) * 512
                                                ],
                                                start=(tk == 0),
                                                stop=(tk == 3),
                                            )
                                        tk += 1
                                for r in range(rq):
                                    tt = tpool.tile([128, 512], f32r, tag="tt")
                                    nc.scalar.activation(
                                        out=tt,
                                        in_=pss[r],
                                        func=mybir.ActivationFunctionType.Tanh,
                                        bias=bias_sb[:, m, b : b + 1],
                                        scale=1.0,
                                    )
                                    tts_r[r][m] = tt
                            for p in pending:
                                emit_vdot(p)
                            pending = []
                            for r in range(rq):
                                vp = vpsum.tile([BPC, 512], f32, tag="vp")
                                pending.append((b, q * rq + r, vp, tts_r[r]))
                    else:
                        for n in range(NCH):
                            q, r = divmod(n, NCH // nq)
                            tts = []
                            for m in range(MM):
                                ps = mpsum.tile([128, 512], f32, tag="ps")
                                i = 0
                                for t in range(2):
                                    for kk in range(KK):
                                        nc.tensor.matmul(
                                            ps,
                                            lhsT=wt_sb[:, t * 4 + kk * 2 + m, :],
                                            rhs=xq[q][:, t, kk, r * 512 : (r + 1) * 512],
                                            start=(i == 0),
                                            stop=(i == 3),
                                        )
                                        i += 1
                                tt = tpool.tile([128, 512], f32r, tag="tt")
                                nc.scalar.activation(
                                    out=tt,
                                    in_=ps,
                                    func=mybir.ActivationFunctionType.Tanh,
                                    bias=bias_sb[:, m, b : b + 1],
                                    scale=1.0,
                                )
                                tts.append(tt)
                            lag = 2 if variant == "lag2" else 1
                            while len(pending) >= lag:
                                emit_vdot(pending.pop(0))
                            pending.append(
                                (
                                    b,
                                    n,
                                    vpsum.tile([BPC, 512], f32, tag="vp", name="vp"),
                                    tts,
                                )
                            )
                # flush the last batch's final v-dots (and exps) after the loop
                for p in pending:
                    emit_vdot(p)
                pending = []

                # Softmax normalization: per-batch sum of the chunk exp-sums,
                # reciprocal, then scale.  The multiplies are split across
                # DVE / ACT / GPSIMD so the tail is short.
                bsum = spool.tile([BPC, 1], f32, tag="bsum")
                nc.vector.reduce_sum(out=bsum, in_=esums, axis=mybir.AxisListType.X)
                recip = spool.tile([BPC, 1], f32, tag="recip")
                nc.vector.reciprocal(out=recip, in_=bsum)
                nc.vector.tensor_scalar_mul(
                    out=scores8[:, 0:2048], in0=scores8[:, 0:2048], scalar1=recip
                )
                nc.scalar.activation(
                    out=scores8[:, 2048:3328],
                    in_=scores8[:, 2048:3328],
                    func=mybir.ActivationFunctionType.Copy,
                    scale=recip,
                )
                nc.gpsimd.tensor_scalar_mul(
                    out=scores8[:, 3328:4096], in0=scores8[:, 3328:4096], scalar1=recip
                )
                # Output DMA rides the (idle) gpsimd queue so its wait on the
                # epilogue never blocks the next rep's input stream on the
                # sync queue.
                nc.gpsimd.dma_start(out=out[:, :], in_=scores8)

    nc.finalize()
    return nc


def prep_shared_inputs(W: np.ndarray, v: np.ndarray, decoder_hidden: np.ndarray):
    """Host-side layout marshaling of the small replicated parameters."""
    W = np.ascontiguousarray(W, dtype=np.float32)
    wt_tiles = np.empty((128, 12, 128), np.float32)
    for t in range(3):
        Wt = W[:, t * H : (t + 1) * H].T  # [k, h]
        for kk in range(KK):
            for m in range(MM):
                j = t * 4 + kk * 2 + m
                wt_tiles[:, j, :] = Wt[kk * 128 : (kk + 1) * 128, m * 128 : (m + 1) * 128]
    vt = np.ascontiguousarray(v[0].reshape(KK, 128).T, dtype=np.float32)  # [p, m]
    vm_tiles = np.zeros((128, MM, BPC, BPC), np.float32)
    for b in range(BPC):
        vm_tiles[:, :, b, b] = vt
    hT = decoder_hidden[0].T.astype(np.float32)  # [H, B]
    return wt_tiles.astype(BF16), vm_tiles, hT


_CACHED = {}


def _get_nc(reps: int = 1, loop_iters: int = 0, nq: int = NQ, variant: str = "base"):
    key = (reps, loop_iters, nq, variant)
    if key not in _CACHED:
        _CACHED[key] = build_bass(reps, loop_iters, nq, variant)
    return _CACHED[key]


def make_in_maps(static_enc, dynamic_enc, decoder_hidden, W, v):
    wt_tiles, vm_tiles, hT = prep_shared_inputs(W, v, decoder_hidden)
    static_enc = np.ascontiguousarray(static_enc, dtype=np.float32)
    dynamic_enc = np.ascontiguousarray(dynamic_enc, dtype=np.float32)
    in_maps = []
    for c in range(N_CORES):
        b0 = c * BPC
        ht_c = np.ascontiguousarray(
            hT[:, b0 : b0 + BPC].reshape(KK, 128, BPC).transpose(1, 0, 2)
        ).astype(BF16)  # [p, kk, b]
        xc_c = np.ascontiguousarray(
            np.stack(
                [static_enc[b0 : b0 + BPC], dynamic_enc[b0 : b0 + BPC]], axis=1
            ).astype(BF16)
        )  # [b, t, h, s]
        in_maps.append(
            {
                "xc": xc_c,
                "wt": wt_tiles,
                "ht": ht_c,
                "vm": vm_tiles,
            }
        )
    return in_maps


def kernel(static_enc, dynamic_enc, decoder_hidden, W, v):
    from concourse.bass_utils import run_bass_kernel_spmd

    nc = _get_nc(reps=1)
    in_maps = make_in_maps(static_enc, dynamic_enc, decoder_hidden, W, v)
    res = run_bass_kernel_spmd(nc, in_maps, core_ids=list(range(N_CORES)))
    return np.concatenate([r["out"] for r in res.results], axis=0)


# revision 31
# speedup vs baseline: 1.4236x; 1.0484x over previous
"""Trainium2 Bass kernel for the sparse-attention scores module.

Computes, for each batch b:
    scores[b, :] = softmax_s( v . tanh(W1 @ static[b] + W2 @ dynamic[b] + W3 @ hidden[b]) )
with W = [W1 | W2 | W3] of shape [H, 3H], static/dynamic [B, H, S], hidden [B, H].

Sharding: data-parallel over B across 8 NeuronCores (8 batches per core).
Per core the kernel streams 32 MiB of host-cast bf16 encoder data from HBM,
runs the two [256,256]@[256,4096] matmuls per batch in bf16 on the PE
(fp32 PSUM), adds the per-batch bias and applies tanh on the ACT engine,
and reduces with v via masked-lhsT PE matmuls (column b of the lhsT holds v,
so batch b's scores land directly on PSUM partition b).  The bf16 staging
costs ~1.4e-3 rel-l2 (gate: 2e-2); with it the kernel is PE-bound (~137 us
of matmul per pass) rather than HBM-bound.

Two v-dot layouts:
 - "base": per-chunk [8,512] v-dot PSUM tiles merged into a [8,4096] SBUF
   scores tile by DVE add; per-partition softmax epilogue.
 - "pack4": chunk n's v-dot writes PSUM partitions 32*(n%4)+b of one of two
   long-lived [128,512] accumulator banks (column slot n//4), accumulating
   over batches in PSUM.  The four 32-column PE-array groups let packed
   v-dots overlap in flight (tile_position is derived from the output
   partition base), and the DVE merge disappears entirely.
"""

import sys

sys.path.insert(0, "/opt/trn_rl_repo")

import ml_dtypes
import numpy as np

BF16 = ml_dtypes.bfloat16

B, H, S = 64, 256, 4096
N_CORES = 8
BPC = B // N_CORES          # batches per core
KK = H // 128               # 2 contraction chunks
MM = H // 128               # 2 output-row chunks
NCH = S // 512              # 8 psum column chunks
NQ = 4                      # input DMA quarters along s
SQ = S // NQ                # 1024 columns per quarter
NG = 3                      # pack4: usable PE-array column groups (quadrant 3 is off-limits)
NJ = -(-NCH // NG)          # pack4: column slots per scores row


def build_bass(reps: int = 1, loop_iters: int = 0, nq: int = NQ, variant: str = "pack4"):
    """Build the per-core Bass program. reps>1 unrolls the whole computation
    multiple times; loop_iters>0 additionally wraps the unrolled body in a
    hardware loop. Both are used only for timing by differencing."""
    import contextlib

    import concourse.bacc as bacc
    import concourse.tile as tile
    from concourse import mybir

    f32 = mybir.dt.float32
    f32r = mybir.dt.float32r
    bf16 = mybir.dt.bfloat16

    nc = bacc.Bacc(None)

    # Encoder streams (the HBM bottleneck) and the tiny W/hidden params are
    # staged as bf16 (host-cast).  tanh outputs and the v-dot stay fp32.
    xc = nc.dram_tensor("xc", [BPC, 2, H, S], bf16, kind="ExternalInput")
    wt = nc.dram_tensor("wt", [128, 12, 128], bf16, kind="ExternalInput")
    ht = nc.dram_tensor("ht", [128, KK, BPC], bf16, kind="ExternalInput")
    vm = nc.dram_tensor("vm", [128, MM, BPC, BPC], bf16, kind="ExternalInput")
    out = nc.dram_tensor("out", [BPC, S], f32, kind="ExternalOutput")

    sq = S // nq
    with tile.TileContext(nc) as tc:
        with (
            tc.tile_pool(name="consts", bufs=1) as consts,
            tc.tile_pool(name="xpool", bufs=6) as xpool,
            tc.tile_pool(name="tpool", bufs=(12 if variant == "pack4" else 6)) as tpool,
            tc.tile_pool(name="spool", bufs=2) as spool,
            tc.tile_pool(name="mpsum", bufs=4, space="PSUM") as mpsum,
            tc.tile_pool(
                name="vpsum", bufs=(1 if variant == "pack4" else 2), space="PSUM"
            ) as vpsum,
            tc.tile_pool(
                name="spsum", bufs=(1 if variant == "pack4" else 2), space="PSUM"
            ) as spsum,
        ):
            wt_sb = consts.tile([128, 12, 128], bf16)
            nc.sync.dma_start(out=wt_sb, in_=wt[:, :, :])
            ht_sb = consts.tile([128, KK, BPC], bf16)
            nc.sync.dma_start(out=ht_sb, in_=ht[:, :, :])
            vm_sb = consts.tile([128, MM, BPC, BPC], bf16)
            nc.sync.dma_start(out=vm_sb, in_=vm[:, :, :, :])

            if variant == "pack4":
                # esmask[32c+b, b] = 1: per-batch total of the 4 group
                # exp-sums.  bcmask[b, 32c+b] = 1: broadcast recip back to
                # the 4 partition groups.
                es_np = np.zeros((128, BPC), np.float32)
                bc_np = np.zeros((BPC, 128), np.float32)
                for c in range(NG):
                    for bb in range(BPC):
                        es_np[32 * c + bb, bb] = 1.0
                        bc_np[bb, 32 * c + bb] = 1.0
                es_dram = nc.inline_tensor(es_np, name="esm")
                bc_dram = nc.inline_tensor(bc_np, name="bcm")
                esmask_sb = consts.tile([128, BPC], f32)
                nc.sync.dma_start(out=esmask_sb, in_=es_dram[:, :])
                bcmask_sb = consts.tile([BPC, 128], f32)
                nc.sync.dma_start(out=bcmask_sb, in_=bc_dram[:, :])

            # Per-batch bias: bias[m*128+h', b] = (W3 @ hidden[b])[m*128+h']
            bias_sb = consts.tile([128, MM, BPC], f32)
            for m in range(MM):
                bias_ps = spsum.tile([128, BPC], f32, tag="small")
                for kk in range(KK):
                    nc.tensor.matmul(
                        bias_ps,
                        lhsT=wt_sb[:, 8 + kk * 2 + m, :],
                        rhs=ht_sb[:, kk, :],
                        start=(kk == 0),
                        stop=(kk == KK - 1),
                    )
                nc.vector.tensor_copy(out=bias_sb[:, m, :], in_=bias_ps)

            loop_cm = (
                tc.For_i(0, loop_iters, 1) if loop_iters else contextlib.nullcontext()
            )
            with loop_cm:
              for _ in range(reps):
                if variant == "pack4":
                    # scores[32c+b, 512j+s] = batch b's chunk n=4j+c.
                    scores = spool.tile([128, NJ * 512], f32, tag="scores")
                    esums = spool.tile([128, NJ], f32, tag="esums")
                    # Rows 8..31 of each partition group are never written by
                    # the exps; zero them so the mask matmuls and normalize
                    # multiplies never touch stray NaN/Inf bit patterns.
                    nc.vector.memset(scores, 0.0)
                    nc.vector.memset(esums, 0.0)
                    vpj = [
                        vpsum.tile([128, 512], f32, tag=f"vpj{j}", name=f"vpj{j}")
                        for j in range(NJ)
                    ]
                else:
                    scores8 = spool.tile([BPC, S], f32, tag="scores")
                    esums8 = spool.tile([BPC, NCH], f32, tag="esums")
                pending = []

                def emit_vdot(pend):
                    # base: v-dot one chunk late; [8,512] PSUM merged by DVE.
                    b, n, vp, tts = pend
                    for m in range(MM):
                        nc.tensor.matmul(
                            vp,
                            lhsT=vm_sb[:, m, b, :],
                            rhs=tts[m],
                            start=(m == 0),
                            stop=(m == MM - 1),
                        )
                    sl = scores8[:, n * 512 : (n + 1) * 512]
                    if b == 0:
                        nc.vector.tensor_copy(out=sl, in_=vp)
                    else:
                        nc.vector.tensor_add(out=sl, in0=vp, in1=sl)
                    if b == BPC - 1:
                        nc.scalar.activation(
                            out=sl,
                            in_=sl,
                            func=mybir.ActivationFunctionType.Exp,
                            accum_out=esums8[:, n : n + 1],
                        )

                def emit_group(chunks):
                    # pack4: consecutive chunks' v-dots back-to-back.  Each
                    # writes out partitions 32c..32c+8, i.e. a distinct
                    # 32-column PE-array group, so the streams overlap in
                    # flight.  Accumulation over batches happens in PSUM
                    # (start at b==0, stop at b==7); rows != b contribute
                    # exact zeros from the masked lhsT.
                    for b, n, tts in chunks:
                        c, j = n % NG, n // NG
                        outap = vpj[j][32 * c : 32 * c + BPC, :]
                        for m in range(MM):
                            nc.tensor.matmul(
                                outap,
                                lhsT=vm_sb[:, m, b, :],
                                rhs=tts[m],
                                start=(b == 0 and m == 0),
                                stop=(b == BPC - 1 and m == MM - 1),
                                skip_group_check=True,
                            )
                    for b, n, tts in chunks:
                        if b == BPC - 1:
                            c, j = n % NG, n // NG
                            sl = scores[
                                32 * c : 32 * c + BPC, 512 * j : 512 * (j + 1)
                            ]
                            nc.scalar.activation(
                                out=sl,
                                in_=vpj[j][32 * c : 32 * c + BPC, :],
                                func=mybir.ActivationFunctionType.Exp,
                                accum_out=esums[32 * c : 32 * c + BPC, j : j + 1],
                            )

                for b in range(BPC):
                    # Stream both encoder tensors in 2 MiB quarters so the
                    # PE can start each pair of column chunks as soon as its
                    # slice of data lands.
                    xq = []
                    for q in range(nq):
                        xt = xpool.tile([128, 2, KK, sq], bf16, tag="xq")
                        src = xc[b, :, :, q * sq : (q + 1) * sq].rearrange(
                            "t (kk p) s -> p t kk s", p=128
                        )
                        nc.sync.dma_start(out=xt, in_=src)
                        xq.append(xt)

                    for n in range(NCH):
                        q, r = divmod(n, NCH // nq)
                        tts = []
                        for m in range(MM):
                            ps = mpsum.tile([128, 512], f32, tag="ps")
                            i = 0
                            for t in range(2):
                                for kk in range(KK):
                                    nc.tensor.matmul(
                                        ps,
                                        lhsT=wt_sb[:, t * 4 + kk * 2 + m, :],
                                        rhs=xq[q][:, t, kk, r * 512 : (r + 1) * 512],
                                        start=(i == 0),
                                        stop=(i == 3),
                                    )
                                    i += 1
                            tt = tpool.tile([128, 512], bf16, tag="tt")
                            nc.scalar.activation(
                                out=tt,
                                in_=ps,
                                func=mybir.ActivationFunctionType.Tanh,
                                bias=bias_sb[:, m, b : b + 1],
                                scale=1.0,
                            )
                            tts.append(tt)
                        if variant == "pack4":
                            pending.append((b, n, tts))
                            if len(pending) == NG + 1:
                                emit_group(pending[:NG])
                                pending = pending[NG:]
                        else:
                            for p in pending:
                                emit_vdot(p)
                            pending = [
                                (
                                    b,
                                    n,
                                    vpsum.tile([BPC, 512], f32, tag="vp", name="vp"),
                                    tts,
                                )
                            ]
                # flush the final v-dots after the loop
                if variant == "pack4":
                    emit_group(pending)
                    pending = []
                else:
                    for p in pending:
                        emit_vdot(p)
                    pending = []

                if variant == "pack4":
                    # Per-batch sums live at partitions {b, 32+b, 64+b, 96+b}
                    # of esums; two tiny mask matmuls total them and broadcast
                    # the reciprocal back; the normalize multiplies split
                    # across DVE / ACT / GPSIMD.
                    bsum_ps = spsum.tile([BPC, NJ], f32, tag="small")
                    nc.tensor.matmul(
                        bsum_ps, lhsT=esmask_sb, rhs=esums, start=True, stop=True
                    )
                    bsum = spool.tile([BPC, 1], f32, tag="bsum")
                    nc.vector.reduce_sum(
                        out=bsum, in_=bsum_ps, axis=mybir.AxisListType.X
                    )
                    recip = spool.tile([BPC, 1], f32, tag="recip")
                    nc.vector.reciprocal(out=recip, in_=bsum)
                    rb_ps = spsum.tile([128, 1], f32, tag="small")
                    nc.tensor.matmul(
                        rb_ps, lhsT=bcmask_sb, rhs=recip, start=True, stop=True
                    )
                    rb = spool.tile([128, 1], f32, tag="rb")
                    nc.vector.tensor_copy(out=rb, in_=rb_ps)
                    nc.vector.tensor_scalar_mul(
                        out=scores[:, 0:768], in0=scores[:, 0:768], scalar1=rb
                    )
                    nc.scalar.activation(
                        out=scores[:, 768:1216],
                        in_=scores[:, 768:1216],
                        func=mybir.ActivationFunctionType.Copy,
                        scale=rb,
                    )
                    nc.gpsimd.tensor_scalar_mul(
                        out=scores[:, 1216:1536], in0=scores[:, 1216:1536], scalar1=rb
                    )
                    # out[b, (3j + c) * 512 + s] <- scores[32c+b, 512j + s]
                    outv = out[:, :].rearrange("b (n s) -> b n s", n=NCH)
                    for c in range(NG):
                        jc = len(range(c, NCH, NG))
                        nc.gpsimd.dma_start(
                            out=outv[:, c : NCH : NG, :],
                            in_=scores[
                                32 * c : 32 * c + BPC, 0 : 512 * jc
                            ].rearrange("b (j s) -> b j s", j=jc),
                        )
                else:
                    bsum = spool.tile([BPC, 1], f32, tag="bsum")
                    nc.vector.reduce_sum(
                        out=bsum, in_=esums8, axis=mybir.AxisListType.X
                    )
                    recip = spool.tile([BPC, 1], f32, tag="recip")
                    nc.vector.reciprocal(out=recip, in_=bsum)
                    nc.vector.tensor_scalar_mul(
                        out=scores8[:, 0:2048], in0=scores8[:, 0:2048], scalar1=recip
                    )
                    nc.scalar.activation(
                        out=scores8[:, 2048:3328],
                        in_=scores8[:, 2048:3328],
                        func=mybir.ActivationFunctionType.Copy,
                        scale=recip,
                    )
                    nc.gpsimd.tensor_scalar_mul(
                        out=scores8[:, 3328:4096],
                        in0=scores8[:, 3328:4096],
                        scalar1=recip,
                    )
                    nc.gpsimd.dma_start(out=out[:, :], in_=scores8)

    nc.finalize()
    return nc


def prep_shared_inputs(W: np.ndarray, v: np.ndarray, decoder_hidden: np.ndarray):
    """Host-side layout marshaling of the small replicated parameters."""
    W = np.ascontiguousarray(W, dtype=np.float32)
    wt_tiles = np.empty((128, 12, 128), np.float32)
    for t in range(3):
        Wt = W[:, t * H : (t + 1) * H].T  # [k, h]
        for kk in range(KK):
            for m in range(MM):
                j = t * 4 + kk * 2 + m
                wt_tiles[:, j, :] = Wt[kk * 128 : (kk + 1) * 128, m * 128 : (m + 1) * 128]
    vt = np.ascontiguousarray(v[0].reshape(KK, 128).T, dtype=np.float32)  # [p, m]
    vm_tiles = np.zeros((128, MM, BPC, BPC), np.float32)
    for b in range(BPC):
        vm_tiles[:, :, b, b] = vt
    vm_tiles = vm_tiles.astype(BF16)
    hT = decoder_hidden[0].T.astype(np.float32)  # [H, B]
    return wt_tiles.astype(BF16), vm_tiles, hT


_CACHED = {}


def _get_nc(reps: int = 1, loop_iters: int = 0, nq: int = NQ, variant: str = "pack4"):
    key = (reps, loop_iters, nq, variant)
    if key not in _CACHED:
        _CACHED[key] = build_bass(reps, loop_iters, nq, variant)
    return _CACHED[key]


def make_in_maps(static_enc, dynamic_enc, decoder_hidden, W, v):
    wt_tiles, vm_tiles, hT = prep_shared_inputs(W, v, decoder_hidden)
    static_enc = np.ascontiguousarray(static_enc, dtype=np.float32)
    dynamic_enc = np.ascontiguousarray(dynamic_enc, dtype=np.float32)
    in_maps = []
    for c in range(N_CORES):
        b0 = c * BPC
        ht_c = np.ascontiguousarray(
            hT[:, b0 : b0 + BPC].reshape(KK, 128, BPC).transpose(1, 0, 2)
        ).astype(BF16)  # [p, kk, b]
        xc_c = np.ascontiguousarray(
            np.stack(
                [static_enc[b0 : b0 + BPC], dynamic_enc[b0 : b0 + BPC]], axis=1
            ).astype(BF16)
        )  # [b, t, h, s]
        in_maps.append(
            {
                "xc": xc_c,
                "wt": wt_tiles,
                "ht": ht_c,
                "vm": vm_tiles,
            }
        )
    return in_maps


def kernel(static_enc, dynamic_enc, decoder_hidden, W, v):
    from concourse.bass_utils import run_bass_kernel_spmd

    nc = _get_nc(reps=1)
    in_maps = make_in_maps(static_enc, dynamic_enc, decoder_hidden, W, v)
    res = run_bass_kernel_spmd(nc, in_maps, core_ids=list(range(N_CORES)))
    return np.concatenate([r["out"] for r in res.results], axis=0)


# revision 35
# speedup vs baseline: 1.5055x; 1.0575x over previous
"""Trainium2 Bass kernel for the sparse-attention scores module.

Computes, for each batch b:
    scores[b, :] = softmax_s( v . tanh(W1 @ static[b] + W2 @ dynamic[b] + W3 @ hidden[b]) )
with W = [W1 | W2 | W3] of shape [H, 3H], static/dynamic [B, H, S], hidden [B, H].

Sharding: data-parallel over B across 8 NeuronCores (8 batches per core).
Per core the kernel streams 32 MiB of host-cast bf16 encoder data from HBM,
runs the two [256,256]@[256,4096] matmuls per batch in bf16 on the PE
(fp32 PSUM), adds the per-batch bias and applies tanh on the ACT engine,
and reduces with v via masked-lhsT PE matmuls (column b of the lhsT holds v,
so batch b's scores land directly on PSUM partition b).  The bf16 staging
costs ~1.4e-3 rel-l2 (gate: 2e-2); with it the kernel is PE-bound (~137 us
of matmul per pass) rather than HBM-bound.

Two v-dot layouts:
 - "base": per-chunk [8,512] v-dot PSUM tiles merged into a [8,4096] SBUF
   scores tile by DVE add; per-partition softmax epilogue.
 - "pack4" (default): chunk n's v-dot writes PSUM partitions 32*(n%3)+b
   of one of three long-lived [128,512] accumulator banks (column slot
   n//3), accumulating over batches in PSUM.  Distinct 32-column PE-array
   groups (bases 0/32/64; quadrant 3 is unusable) let consecutively issued
   v-dots overlap in flight (tile_position derives from the output
   partition base; bf16 operands -- the f32r weight path rejects
   tile_position in walrus), and the DVE merge disappears entirely.
"""

import sys

sys.path.insert(0, "/opt/trn_rl_repo")

import ml_dtypes
import numpy as np

BF16 = ml_dtypes.bfloat16

# NOTE: walrus's --enable-ldw-opt=true (which would hide the ~57 ns/matmul
# LDWEIGHTS overhead) crashes codegen in visitInstLdweights on this program;
# the repo-pinned =false is a compiler-bug workaround, not a tuning choice.

B, H, S = 64, 256, 4096
N_CORES = 8
BPC = B // N_CORES          # batches per core
KK = H // 128               # 2 contraction chunks
MM = H // 128               # 2 output-row chunks
NCH = S // 512              # 8 psum column chunks
NQ = 4                      # input DMA quarters along s
SQ = S // NQ                # 1024 columns per quarter
NG = 3                      # pack4: usable PE-array column groups (quadrant 3 is off-limits)
NJ = -(-NCH // NG)          # pack4: column slots per scores row


def build_bass(reps: int = 1, loop_iters: int = 0, nq: int = NQ, variant: str = "pack4"):
    """Build the per-core Bass program. reps>1 unrolls the whole computation
    multiple times; loop_iters>0 additionally wraps the unrolled body in a
    hardware loop. Both are used only for timing by differencing."""
    import contextlib

    import concourse.bacc as bacc
    import concourse.tile as tile
    from concourse import mybir

    f32 = mybir.dt.float32
    f32r = mybir.dt.float32r
    bf16 = mybir.dt.bfloat16

    nc = bacc.Bacc(None)

    # Encoder streams (the HBM bottleneck) and the tiny W/hidden params are
    # staged as bf16 (host-cast).  tanh outputs and the v-dot stay fp32.
    xc = nc.dram_tensor("xc", [BPC, 2, H, S], bf16, kind="ExternalInput")
    wt = nc.dram_tensor("wt", [128, 12, 128], bf16, kind="ExternalInput")
    ht = nc.dram_tensor("ht", [128, KK, BPC], bf16, kind="ExternalInput")
    vm = nc.dram_tensor("vm", [128, MM, BPC, BPC], bf16, kind="ExternalInput")
    out = nc.dram_tensor("out", [BPC, S], f32, kind="ExternalOutput")

    sq = S // nq
    with tile.TileContext(nc) as tc:
        with (
            tc.tile_pool(name="consts", bufs=1) as consts,
            tc.tile_pool(name="xpool", bufs=6) as xpool,
            tc.tile_pool(name="tpool", bufs=(12 if variant == "pack4" else 6)) as tpool,
            tc.tile_pool(name="spool", bufs=2) as spool,
            tc.tile_pool(name="mpsum", bufs=4, space="PSUM") as mpsum,
            tc.tile_pool(
                name="vpsum", bufs=(1 if variant == "pack4" else 2), space="PSUM"
            ) as vpsum,
            tc.tile_pool(
                name="spsum", bufs=(1 if variant == "pack4" else 2), space="PSUM"
            ) as spsum,
        ):
            wt_sb = consts.tile([128, 12, 128], bf16)
            nc.sync.dma_start(out=wt_sb, in_=wt[:, :, :])
            ht_sb = consts.tile([128, KK, BPC], bf16)
            nc.sync.dma_start(out=ht_sb, in_=ht[:, :, :])
            vm_sb = consts.tile([128, MM, BPC, BPC], bf16)
            nc.sync.dma_start(out=vm_sb, in_=vm[:, :, :, :])

            if variant == "pack4":
                # esmask[32c+b, b] = 1: per-batch total of the group
                # exp-sums.  bcmask[b, 32c+b] = 1: broadcast recip back to
                # the partition groups.
                es_np = np.zeros((128, BPC), np.float32)
                bc_np = np.zeros((BPC, 128), np.float32)
                for c in range(NG):
                    for bb in range(BPC):
                        es_np[32 * c + bb, bb] = 1.0
                        bc_np[bb, 32 * c + bb] = 1.0
                es_dram = nc.inline_tensor(es_np, name="esm")
                bc_dram = nc.inline_tensor(bc_np, name="bcm")
                esmask_sb = consts.tile([128, BPC], f32)
                nc.sync.dma_start(out=esmask_sb, in_=es_dram[:, :])
                bcmask_sb = consts.tile([BPC, 128], f32)
                nc.sync.dma_start(out=bcmask_sb, in_=bc_dram[:, :])

            # Per-batch bias: bias[m*128+h', b] = (W3 @ hidden[b])[m*128+h']
            bias_sb = consts.tile([128, MM, BPC], f32)
            for m in range(MM):
                bias_ps = spsum.tile([128, BPC], f32, tag="small")
                for kk in range(KK):
                    nc.tensor.matmul(
                        bias_ps,
                        lhsT=wt_sb[:, 8 + kk * 2 + m, :],
                        rhs=ht_sb[:, kk, :],
                        start=(kk == 0),
                        stop=(kk == KK - 1),
                    )
                nc.vector.tensor_copy(out=bias_sb[:, m, :], in_=bias_ps)

            loop_cm = (
                tc.For_i(0, loop_iters, 1) if loop_iters else contextlib.nullcontext()
            )
            with loop_cm:
              for _ in range(reps):
                if variant == "pack4":
                    # scores[32c+b, 512j+s] = batch b's chunk n=3j+c.
                    scores = spool.tile([128, NJ * 512], f32, tag="scores")
                    esums = spool.tile([128, NJ], f32, tag="esums")
                    # Rows 8..31 of each partition group are never written by
                    # the exps; zero them so the mask matmuls and normalize
                    # multiplies never touch stray NaN/Inf bit patterns.
                    nc.vector.memset(scores, 0.0)
                    nc.vector.memset(esums, 0.0)
                    vpj = [
                        vpsum.tile([128, 512], f32, tag=f"vpj{j}", name=f"vpj{j}")
                        for j in range(NJ)
                    ]
                else:
                    scores8 = spool.tile([BPC, S], f32, tag="scores")
                    esums8 = spool.tile([BPC, NCH], f32, tag="esums")
                pending = []

                def emit_vdot(pend):
                    # base: v-dot one chunk late; [8,512] PSUM merged by DVE.
                    b, n, vp, tts = pend
                    for m in range(MM):
                        nc.tensor.matmul(
                            vp,
                            lhsT=vm_sb[:, m, b, :],
                            rhs=tts[m],
                            start=(m == 0),
                            stop=(m == MM - 1),
                        )
                    sl = scores8[:, n * 512 : (n + 1) * 512]
                    if b == 0:
                        nc.vector.tensor_copy(out=sl, in_=vp)
                    else:
                        nc.vector.tensor_add(out=sl, in0=vp, in1=sl)
                    if b == BPC - 1:
                        nc.scalar.activation(
                            out=sl,
                            in_=sl,
                            func=mybir.ActivationFunctionType.Exp,
                            accum_out=esums8[:, n : n + 1],
                        )

                def emit_group(chunks):
                    # pack4: consecutive chunks' v-dots back-to-back.  Each
                    # writes out partitions 32c..32c+8, i.e. a distinct
                    # 32-column PE-array group, so the streams overlap in
                    # flight.  Accumulation over batches happens in PSUM
                    # (start at b==0, stop at b==7); rows != b contribute
                    # exact zeros from the masked lhsT.
                    # Issue all m=0 passes before the m=1 passes: PE matmul
                    # starts are FIFO, and a chunk's m0->m1 pair is serial in
                    # its array cells, so m-major order would gate the other
                    # column groups' (concurrent) streams behind each pair.
                    for m in range(MM):
                        for b, n, tts in chunks:
                            c, j = n % NG, n // NG
                            outap = vpj[j][32 * c : 32 * c + BPC, :]
                            nc.tensor.matmul(
                                outap,
                                lhsT=vm_sb[:, m, b, :],
                                rhs=tts[m],
                                start=(b == 0 and m == 0),
                                stop=(b == BPC - 1 and m == MM - 1),
                                skip_group_check=True,
                            )
                    for b, n, tts in chunks:
                        if b == BPC - 1:
                            c, j = n % NG, n // NG
                            sl = scores[
                                32 * c : 32 * c + BPC, 512 * j : 512 * (j + 1)
                            ]
                            nc.scalar.activation(
                                out=sl,
                                in_=vpj[j][32 * c : 32 * c + BPC, :],
                                func=mybir.ActivationFunctionType.Exp,
                                accum_out=esums[32 * c : 32 * c + BPC, j : j + 1],
                            )

                for b in range(BPC):
                    # Stream both encoder tensors in 2 MiB quarters so the
                    # PE can start each pair of column chunks as soon as its
                    # slice of data lands.
                    xq = []
                    for q in range(nq):
                        xt = xpool.tile([128, 2, KK, sq], bf16, tag="xq")
                        src = xc[b, :, :, q * sq : (q + 1) * sq].rearrange(
                            "t (kk p) s -> p t kk s", p=128
                        )
                        nc.sync.dma_start(out=xt, in_=src)
                        xq.append(xt)

                    for n in range(NCH):
                        q, r = divmod(n, NCH // nq)
                        tts = []
                        for m in range(MM):
                            ps = mpsum.tile([128, 512], f32, tag="ps")
                            i = 0
                            for t in range(2):
                                for kk in range(KK):
                                    nc.tensor.matmul(
                                        ps,
                                        lhsT=wt_sb[:, t * 4 + kk * 2 + m, :],
                                        rhs=xq[q][:, t, kk, r * 512 : (r + 1) * 512],
                                        start=(i == 0),
                                        stop=(i == 3),
                                    )
                                    i += 1
                            tt = tpool.tile([128, 512], bf16, tag="tt")
                            nc.scalar.activation(
                                out=tt,
                                in_=ps,
                                func=mybir.ActivationFunctionType.Tanh,
                                bias=bias_sb[:, m, b : b + 1],
                                scale=1.0,
                            )
                            tts.append(tt)
                        if variant == "pack4":
                            pending.append((b, n, tts))
                            if len(pending) == NG + 1:
                                emit_group(pending[:NG])
                                pending = pending[NG:]
                        else:
                            for p in pending:
                                emit_vdot(p)
                            pending = [
                                (
                                    b,
                                    n,
                                    vpsum.tile([BPC, 512], f32, tag="vp", name="vp"),
                                    tts,
                                )
                            ]
                # flush the final v-dots after the loop
                if variant == "pack4":
                    emit_group(pending)
                    pending = []
                else:
                    for p in pending:
                        emit_vdot(p)
                    pending = []

                if variant == "pack4":
                    # Per-batch sums live at partitions {b, 32+b, 64+b, 96+b}
                    # of esums; two tiny mask matmuls total them and broadcast
                    # the reciprocal back; the normalize multiplies split
                    # across DVE / ACT / GPSIMD.
                    bsum_ps = spsum.tile([BPC, NJ], f32, tag="small")
                    nc.tensor.matmul(
                        bsum_ps, lhsT=esmask_sb, rhs=esums, start=True, stop=True
                    )
                    bsum = spool.tile([BPC, 1], f32, tag="bsum")
                    nc.vector.reduce_sum(
                        out=bsum, in_=bsum_ps, axis=mybir.AxisListType.X
                    )
                    recip = spool.tile([BPC, 1], f32, tag="recip")
                    nc.vector.reciprocal(out=recip, in_=bsum)
                    rb_ps = spsum.tile([128, 1], f32, tag="small")
                    nc.tensor.matmul(
                        rb_ps, lhsT=bcmask_sb, rhs=recip, start=True, stop=True
                    )
                    rb = spool.tile([128, 1], f32, tag="rb")
                    nc.vector.tensor_copy(out=rb, in_=rb_ps)
                    nc.vector.tensor_scalar_mul(
                        out=scores[:, 0:768], in0=scores[:, 0:768], scalar1=rb
                    )
                    nc.scalar.activation(
                        out=scores[:, 768:1216],
                        in_=scores[:, 768:1216],
                        func=mybir.ActivationFunctionType.Copy,
                        scale=rb,
                    )
                    nc.gpsimd.tensor_scalar_mul(
                        out=scores[:, 1216:1536], in0=scores[:, 1216:1536], scalar1=rb
                    )
                    # out[b, (3j + c) * 512 + s] <- scores[32c+b, 512j + s]
                    outv = out[:, :].rearrange("b (n s) -> b n s", n=NCH)
                    for c in range(NG):
                        jc = len(range(c, NCH, NG))
                        nc.gpsimd.dma_start(
                            out=outv[:, c : NCH : NG, :],
                            in_=scores[
                                32 * c : 32 * c + BPC, 0 : 512 * jc
                            ].rearrange("b (j s) -> b j s", j=jc),
                        )
                else:
                    bsum = spool.tile([BPC, 1], f32, tag="bsum")
                    nc.vector.reduce_sum(
                        out=bsum, in_=esums8, axis=mybir.AxisListType.X
                    )
                    recip = spool.tile([BPC, 1], f32, tag="recip")
                    nc.vector.reciprocal(out=recip, in_=bsum)
                    nc.vector.tensor_scalar_mul(
                        out=scores8[:, 0:2048], in0=scores8[:, 0:2048], scalar1=recip
                    )
                    nc.scalar.activation(
                        out=scores8[:, 2048:3328],
                        in_=scores8[:, 2048:3328],
                        func=mybir.ActivationFunctionType.Copy,
                        scale=recip,
                    )
                    nc.gpsimd.tensor_scalar_mul(
                        out=scores8[:, 3328:4096],
                        in0=scores8[:, 3328:4096],
                        scalar1=recip,
                    )
                    nc.gpsimd.dma_start(out=out[:, :], in_=scores8)

    nc.finalize()
    return nc


def prep_shared_inputs(W: np.ndarray, v: np.ndarray, decoder_hidden: np.ndarray):
    """Host-side layout marshaling of the small replicated parameters."""
    W = np.ascontiguousarray(W, dtype=np.float32)
    wt_tiles = np.empty((128, 12, 128), np.float32)
    for t in range(3):
        Wt = W[:, t * H : (t + 1) * H].T  # [k, h]
        for kk in range(KK):
            for m in range(MM):
                j = t * 4 + kk * 2 + m
                wt_tiles[:, j, :] = Wt[kk * 128 : (kk + 1) * 128, m * 128 : (m + 1) * 128]
    vt = np.ascontiguousarray(v[0].reshape(KK, 128).T, dtype=np.float32)  # [p, m]
    vm_tiles = np.zeros((128, MM, BPC, BPC), np.float32)
    for b in range(BPC):
        vm_tiles[:, :, b, b] = vt
    vm_tiles = vm_tiles.astype(BF16)
    hT = decoder_hidden[0].T.astype(np.float32)  # [H, B]
    return wt_tiles.astype(BF16), vm_tiles, hT


_CACHED = {}


def _get_nc(reps: int = 1, loop_iters: int = 0, nq: int = NQ, variant: str = "pack4"):
    key = (reps, loop_iters, nq, variant)
    if key not in _CACHED:
        _CACHED[key] = build_bass(reps, loop_iters, nq, variant)
    return _CACHED[key]


def make_in_maps(static_enc, dynamic_enc, decoder_hidden, W, v):
    wt_tiles, vm_tiles, hT = prep_shared_inputs(W, v, decoder_hidden)
    static_enc = np.ascontiguousarray(static_enc, dtype=np.float32)
    dynamic_enc = np.ascontiguousarray(dynamic_enc, dtype=np.float32)
    in_maps = []
    for c in range(N_CORES):
        b0 = c * BPC
        ht_c = np.ascontiguousarray(
            hT[:, b0 : b0 + BPC].reshape(KK, 128, BPC).transpose(1, 0, 2)
        ).astype(BF16)  # [p, kk, b]
        xc_c = np.ascontiguousarray(
            np.stack(
                [static_enc[b0 : b0 + BPC], dynamic_enc[b0 : b0 + BPC]], axis=1
            ).astype(BF16)
        )  # [b, t, h, s]
        in_maps.append(
            {
                "xc": xc_c,
                "wt": wt_tiles,
                "ht": ht_c,
                "vm": vm_tiles,
            }
        )
    return in_maps


def kernel(static_enc, dynamic_enc, decoder_hidden, W, v):
    from concourse.bass_utils import run_bass_kernel_spmd

    nc = _get_nc(reps=1)
    in_maps = make_in_maps(static_enc, dynamic_enc, decoder_hidden, W, v)
    res = run_bass_kernel_spmd(nc, in_maps, core_ids=list(range(N_CORES)))
    return np.concatenate([r["out"] for r in res.results], axis=0)
